# revision 21
# baseline (speedup 1.0000x reference)
"""Trainium2 Bass kernel for nn_D_loss_67551245631962.

Computes: 0.8 * sum(WMA5(target_angle - pred_angle)^2) + 0.2 * sum((target_class - pred_class)^2)
where WMA5 is a 5-tap [0.05, 0.1, 0.7, 0.1, 0.05] correlation with 2-zero padding per side.

Strategy (data parallel over batch B=2048 across 8 cores; 256 rows = 2 row-groups/core):
  Work is cut into 8 independent 2048-column chunks per core (4 per row-group),
  each with a 2-col halo on both sides (edge halos zeroed).

  The 5-tap conv runs on the idle TensorEngine as shifted-slice matmuls:
      psum[:, b*512:(b+1)*512] += (w_k * I)^T @ d[:, b*512+k : b*512+k+512]
  accumulated over taps k=0..4 into PSUM (fp32, 4 banks per chunk), with the
  scaled identities preloaded in SBUF. ScalarE then does Square + accum_out
  DIRECTLY from PSUM into an fp32 accumulator column. Per chunk:
      d = ta - pa      DVE TT fp16 (cast loads), 1.2us
      conv             PE, 20 matmuls (5 taps x 4 banks), ~4.8us
      sum(s^2)         ACT Square from PSUM + accum col, ~2.0us
  Loads are SWDGE cast DMAs (fp32 DRAM -> fp16 SBUF) on the single Q7 FIFO
  queue: chunks arrive in issue order every ~4.9us, and each engine's
  per-chunk work is below that pace. The last chunk is processed per-bank
  (512 cols) to shorten the post-DMA tail. Host applies 0.8*0.05^2 / 0.2.
"""

import os
import sys

for _p in ("/opt/trn_rl_repo",):
    if os.path.isdir(_p) and _p not in sys.path:
        sys.path.insert(0, _p)

from contextlib import ExitStack

import numpy as np

import concourse.bass as bass
import concourse.tile as tile
from concourse import bacc, mybir
from concourse.bass_utils import run_bass_kernel_spmd
from concourse.masks import make_identity

N_CORES = 8
B, T = 2048, 8192
RPC = B // N_CORES  # rows per core = 256
G = RPC // 128      # row groups per core = 2
W = 2048            # chunk width
NJ = T // W         # chunks per group = 4
BW = 512            # psum bank width (fp32)
NB = W // BW        # banks per chunk = 4

DT16 = mybir.dt.float16
W5 = (1.0, 2.0, 14.0, 2.0, 1.0)

# units: (g, c0, width) — full-width chunks for j=0..2, the last 2048 cols
# split into 1024-wide units so the post-DMA tail pipeline is short
UNITS = [(g, j * W, W) for j in range(NJ - 1) for g in range(G)] + [
    (g, (NJ - 1) * W + h * (W // 2), W // 2) for h in range(2) for g in range(G)
]
NU = len(UNITS)  # 10

PW = 1024           # conv/square processing width (2 psum banks)
NANG = T // PW * G  # one accum col per 1024-col block = 16
NACC = NANG + G


def build_nc():
    nc = bacc.Bacc("TRN2")
    dt = mybir.dt
    ta = nc.dram_tensor("target_angle", [RPC, T], dt.float32, kind="ExternalInput")
    pa = nc.dram_tensor("pred_angle", [RPC, T], dt.float32, kind="ExternalInput")
    tcl = nc.dram_tensor("target_class", [RPC, 3], dt.float32, kind="ExternalInput")
    pcl = nc.dram_tensor("pred_class", [RPC, 3], dt.float32, kind="ExternalInput")
    out = nc.dram_tensor("out", [128, NACC], dt.float32, kind="ExternalOutput")

    AF = mybir.ActivationFunctionType
    OP = mybir.AluOpType

    with tile.TileContext(nc) as tc, ExitStack() as ctx:
        apool = ctx.enter_context(tc.tile_pool(name="la", bufs=8))
        bpool = ctx.enter_context(tc.tile_pool(name="lb", bufs=8))
        dpool = ctx.enter_context(tc.tile_pool(name="d", bufs=4))
        jpool = ctx.enter_context(tc.tile_pool(name="junk", bufs=3))
        kpool = ctx.enter_context(tc.tile_pool(name="acc", bufs=1))
        wpool = ctx.enter_context(tc.tile_pool(name="wid", bufs=1))
        cpool = ctx.enter_context(tc.tile_pool(name="cls", bufs=2))
        psum = ctx.enter_context(tc.psum_pool(name="ps", bufs=4))

        accums = kpool.tile([128, NACC], dt.float32)

        # scaled identities for the 5 taps (fp16, exact small ints)
        ident = wpool.tile([128, 128], DT16)
        make_identity(nc, ident[:])
        wid = wpool.tile([128, 5 * 128], DT16)
        for k in range(5):
            nc.vector.tensor_scalar_mul(wid[:, k * 128 : (k + 1) * 128], ident[:], W5[k])

        # class loads first on the sync ring (tiny)
        cls_tiles = []
        for g in range(G):
            r0, r1 = g * 128, (g + 1) * 128
            ct = cpool.tile([128, 3], dt.float32, tag="ct")
            cp = cpool.tile([128, 3], dt.float32, tag="cp")
            nc.sync.dma_start(ct[:], tcl[r0:r1, :])
            nc.sync.dma_start(cp[:], pcl[r0:r1, :])
            cls_tiles.append((ct, cp))

        # Phase A: all SWDGE cast loads in unit order
        loads = {}
        for (g, c0, wu) in UNITS:
            r0, r1 = g * 128, (g + 1) * 128
            lo, hi = c0 - 2, c0 + wu + 2
            pad_l = pad_r = 0
            if lo < 0:
                pad_l, lo = -lo, 0
            if hi > T:
                pad_r, hi = hi - T, T
            wdt = hi - lo
            A = apool.tile([128, wu + 4], DT16, tag=f"A{wu}")
            Bt = bpool.tile([128, wu + 4], DT16, tag=f"B{wu}")
            if pad_l:
                nc.vector.memset(A[:, 0:pad_l], 0.0)
                nc.vector.memset(Bt[:, 0:pad_l], 0.0)
            if pad_r:
                nc.vector.memset(A[:, pad_l + wdt :], 0.0)
                nc.vector.memset(Bt[:, pad_l + wdt :], 0.0)
            nc.gpsimd.dma_start(A[:, pad_l : pad_l + wdt], ta[r0:r1, lo:hi])
            nc.gpsimd.dma_start(Bt[:, pad_l : pad_l + wdt], pa[r0:r1, lo:hi])
            loads[(g, c0)] = (A, Bt)

        # Phase B: per unit: DVE sub -> PE conv -> ACT square-from-PSUM
        next_col = [0]

        # class SSE early (inputs land in the first microseconds)
        for g in range(G):
            ct, cp = cls_tiles[g]
            cd = cpool.tile([128, 3], dt.float32, tag="cd")
            nc.vector.tensor_sub(cd[:], ct[:], cp[:])
            cj = cpool.tile([128, 3], dt.float32, tag="cj")
            col = NANG + g
            nc.scalar.activation(
                cj[:], cd[:], AF.Square, accum_out=accums[:, col : col + 1]
            )

        def emit_conv_sq(d, doff, wu):
            # one 1024-col block: 10 matmuls into a 2-bank psum tile + 1 square
            ps = psum.tile([128, PW], mybir.dt.float32)
            for k in range(5):
                for b in range(wu // BW):
                    nc.tensor.matmul(
                        ps[:, b * BW : (b + 1) * BW],
                        wid[:, k * 128 : (k + 1) * 128],
                        d[:, doff + b * BW + k : doff + b * BW + k + BW],
                        start=(k == 0),
                        stop=(k == 4),
                    )
            col = next_col[0]
            next_col[0] += 1
            junk = jpool.tile([128, wu], DT16, tag="junk")
            nc.scalar.activation(
                junk[:],
                ps[:, 0:wu],
                AF.Square,
                accum_out=accums[:, col : col + 1],
            )

        for (g, c0, wu) in UNITS:
            A, Bt = loads[(g, c0)]
            d_tile = dpool.tile([128, wu + 4], DT16, tag=f"d{wu}")
            nc.vector.tensor_sub(d_tile[:], A[:], Bt[:])
            for off in range(0, wu, PW):
                emit_conv_sq(d_tile, off, min(PW, wu - off))

        nc.sync.dma_start(out[:], accums[:])

    nc.finalize()
    return nc


_NC = None
last_result = None  # BassKernelResults of the most recent run (for test harness)


def kernel(target_angle, pred_angle, target_class, pred_class):
    global _NC, last_result
    if _NC is None:
        _NC = build_nc()

    in_maps = []
    for c in range(N_CORES):
        r = slice(c * RPC, (c + 1) * RPC)
        in_maps.append(
            {
                "target_angle": np.ascontiguousarray(target_angle[r], dtype=np.float32),
                "pred_angle": np.ascontiguousarray(pred_angle[r], dtype=np.float32),
                "target_class": np.ascontiguousarray(target_class[r], dtype=np.float32),
                "pred_class": np.ascontiguousarray(pred_class[r], dtype=np.float32),
            }
        )

    last_result = run_bass_kernel_spmd(
        _NC,
        in_maps,
        core_ids=list(range(N_CORES)),
        trace=bool(os.environ.get("BASS_TRACE")),
    )

    angle = 0.0
    cls = 0.0
    for r in last_result.results:
        o = np.asarray(r["out"], dtype=np.float64)
        angle += o[:, 0:NANG].sum()
        cls += o[:, NANG:NACC].sum()

    val = 0.8 * (0.05 * 0.05) * angle + 0.2 * cls
    return np.array(val, dtype=np.float32)
